# revision 18
# baseline (speedup 1.0000x reference)
"""Trainium2 Bass kernel for nn_Conv2dGeneral (capsule-style 4x4-pose conv).

Math (from the reference):
  out[b,o,X,Y,u,w] = sum_{cin,kx,ky,v} Wm[(cin,kx,ky),o,u,v] * x[b,cin,2X+kx,2Y+ky,4v+w] + bias[o]

Mapped to the PE array as a single 1152-deep contraction:
  K = (cin, v)  x  9 accumulation chunks over (kx, ky)   [9 x 128]
  M = (o, u)                                              [128 PSUM partitions]
  N = (X, Y, w)                                           [<=676 per batch]

Data-parallel across 8 NeuronCores on the batch dim (8 images per core).

Host-side prep: x is re-laid-out to [(b), (cin,v), (r,c,w)] with the unused
row/col 27 trimmed (stride-2 K=3 windows over 28 only touch 0..26), so each
core's shard DMAs as contiguous 5.8KB partition lines; the im2col window
gather happens for free inside the matmul moving-operand access pattern.

Scheduling model (measured): the core clock runs at HALF speed except for a
single ~23.9us full-speed budget granted ~7us after the DMA stream starts
(pausing if the PE idles), and the early DMA stream only manages
~150-230GB/s (per-packet overhead ~90ns).  So the critical path is "when
can the PE start" plus "does the post-kernel semaphore sweep still fit in
the full-speed window".  The kernel therefore splits the load across BOTH
HWDGE queues: the SP queue carries weight taps 0-2 then pure x in PE
consumption order (batch 0 in three row-chunks so the first small PSUM
group starts ~2us earlier), while the Scalar queue's ~2us cold-start
swallows weight taps 3-8, which are only needed mid-way into the first
accumulation group.  ACT pre-warms its lazy activation-table load with a
dummy 1-element activation, evicts PSUM->SBUF in fp16, and ships each
X-half right after its eviction so the final drain is one 80KB chunk.

The bias add lives on the host (a 128-descriptor 4-byte-line DMA clogs the
queue for ~1.2us).

Each DMA gets its OWN semaphore and consumers wait for the full +16: the 16
increments of one DMA are sem-update packets distributed round-robin over
the 16 DMA engines in engine-local order, so two DMAs sharing a semaphore
can satisfy a >=16 wait with a mix of packets from both while a slow engine
still has data of the first in flight (cold-first-run corruption).
"""

import numpy as np

B, CIN, COUT = 64, 32, 32
KK, STRIDE = 3, 2
WIN, HH = 28, 16
H = 4
WOUT = (WIN - KK) // STRIDE + 1  # 13
NCORES = 8
BPC = B // NCORES                # batches per core
RR = 2 * WOUT + 1                # 27 rows/cols actually read
RCW = RR * RR * H                # 2916 free elements per (cin,v) partition
NOUT = WOUT * WOUT * H           # 676 outputs per (o,u) partition per image

_cache = {}


def _build_bass():
    """Raw-bass build (no Tile): this toolchain's walrus codegen allows only
    ONE sync-wait per instruction, so all cross-engine sync is explicit
    single-sem waits; ordering beyond that rides on hardware transitivity."""
    import concourse.bass as bass
    import concourse.mybir as mybir
    from contextlib import ExitStack

    f32 = mybir.dt.float32
    f16 = mybir.dt.float16
    WARMUP = 3                # PE warm-up matmuls while x[0] streams in
    HC = WOUT * H             # 52 output columns per X row
    W0E = 3 * 128             # weight taps 0..2 (warmup + first matmuls)
    N0 = 7 * HC               # X-half output widths: 364 / 312
    # batch-0 row chunks: rows [0,9) [9,15) [15,27) feeding X groups
    # [0,4) [4,7) [7,13)  (chunk boundaries reuse earlier rows in SBUF)
    R0E = 9 * RR * H
    R1E = 15 * RR * H

    nc = bass.Bass()
    x_d = nc.declare_dram_parameter("x", [BPC, 128, RCW], f16, isOutput=False)
    w_d = nc.declare_dram_parameter("w", [128, 9 * 128], f16, isOutput=False)
    o_d = nc.declare_dram_parameter("out", [BPC, 128, NOUT], f16, isOutput=True)

    with ExitStack() as stack:
        ec = stack.enter_context
        wt = ec(nc.sbuf_tensor([128, 9 * 128], f16))
        gt = ec(nc.sbuf_tensor([128, BPC, RCW], f16))
        ot = ec(nc.sbuf_tensor([128, BPC, NOUT], f16))
        ps = ec(nc.psum_tensor([128, 8, 512], f32))
        wt0_sem = ec(nc.semaphore("wt0_sem"))
        wtr_sem = ec(nc.semaphore("wtr_sem"))
        c_sems = [ec(nc.semaphore(f"c_sem{i}")) for i in range(3)]
        g_sems = {b: ec(nc.semaphore(f"g_sem{b}")) for b in range(1, BPC)}
        pe_sem = ec(nc.semaphore("pe_sem"))
        act_sem = ec(nc.semaphore("act_sem"))
        out_sem = ec(nc.semaphore("out_sem"))
        block = ec(nc.Block())
        wtr = wt[:, :].rearrange("p (k m) -> p k m", k=9)

        # (batch, X0, nX, input-gate sem): batch 0 in three X chunks, the
        # rest in two X halves
        groups = [(0, 0, 4, c_sems[0]), (0, 4, 3, c_sems[1]), (0, 7, 6, c_sems[2])]
        for b in range(1, BPC):
            groups.append((b, 0, 7, g_sems[b]))
            groups.append((b, 7, 6, None))
        NG = len(groups)  # 17

        @block.sync
        def _(sync):
            # SP queue: pure x in PE consumption order (weights ride the
            # Scalar queue in parallel)
            sync.dma_start(gt[:, 0, :R0E], x_d[0, :, :R0E]).then_inc(c_sems[0], 16)
            sync.dma_start(gt[:, 0, R0E:R1E], x_d[0, :, R0E:R1E]).then_inc(
                c_sems[1], 16
            )
            sync.dma_start(gt[:, 0, R1E:], x_d[0, :, R1E:]).then_inc(c_sems[2], 16)
            for b in range(1, BPC):
                sync.dma_start(gt[:, b, :], x_d[b]).then_inc(g_sems[b], 16)
            sync.wait_ge(out_sem, 16 * 2 * BPC)

        @block.tensor
        def _(tensor):
            tensor.wait_ge(wt0_sem, 16)
            # keep PE visibly busy from the start (pulls the clock grant in)
            for i in range(WARMUP):
                tensor.matmul(
                    ps[:, 7, :128], wt[:, :128], wt[:, :128], start=True, stop=True
                )
            for j, (b, X0, nX, gate) in enumerate(groups):
                if gate is not None:
                    tensor.wait_ge(gate, 16)
                if j >= 8:
                    # PSUM bank j%8 is free once group j-8 was evicted
                    tensor.wait_ge(act_sem, j - 7)
                gr = gt[:, b, :].rearrange("p (r c w) -> p r c w", r=RR, c=RR)
                for kk in range(9):
                    if j == 0 and kk == 3:
                        tensor.wait_ge(wtr_sem, 16)  # taps 3-8 land mid-group
                    kx, ky = divmod(kk, 3)
                    rhs = gr[
                        :,
                        2 * X0 + kx : 2 * X0 + kx + 2 * nX - 1 : 2,
                        ky : ky + 2 * WOUT - 1 : 2,
                        :,
                    ]
                    mm = tensor.matmul(
                        ps[:, j % 8, : nX * HC],
                        wtr[:, kk, :],
                        rhs,
                        start=(kk == 0),
                        stop=(kk == 8),
                    )
                mm.then_inc(pe_sem, 1)

        @block.scalar
        def _(scalar):
            # Scalar HWDGE queue carries both weight pieces in parallel with
            # the SP-queue x stream; taps 0-2 first (warmup + first matmuls)
            scalar.dma_start(wt[:, :W0E], w_d[:, :W0E]).then_inc(wt0_sem, 16)
            scalar.dma_start(wt[:, W0E:], w_d[:, W0E:]).then_inc(wtr_sem, 16)
            # pre-warm the lazy activation-table load off the critical path
            scalar.wait_ge(wt0_sem, 16)
            scalar.activation(
                ot[:, 0, :1], wt[:, :1], mybir.ActivationFunctionType.Identity
            )
            for j, (b, X0, nX, gate) in enumerate(groups):
                off = X0 * HC
                scalar.wait_ge(pe_sem, j + 1)
                scalar.activation(
                    ot[:, b, off : off + nX * HC],
                    ps[:, j % 8, : nX * HC],
                    mybir.ActivationFunctionType.Identity,
                ).then_inc(act_sem, 1)
                # ship each completed X-half right after its eviction
                if X0 + nX == 7:
                    scalar.dma_start(
                        o_d[b, :, :N0], ot[:, b, :N0]
                    ).then_inc(out_sem, 16)
                elif X0 + nX == WOUT:
                    scalar.dma_start(
                        o_d[b, :, N0:], ot[:, b, N0:]
                    ).then_inc(out_sem, 16)

    return nc


def _prep_inputs(x, W, bias):
    # x: (B, CIN, 28, 28, 16) -> xp[b, cin*4+v, (r*27+c)*4+w] = x[b,cin,r,c,4v+w]
    # (row/col 27 trimmed: stride-2 3-wide windows only read 0..26)
    # fp16 halves the dominant HBM traffic; fp32 PSUM accumulation keeps the
    # 1152-deep contraction at ~5e-4 max rel err.
    xp = np.ascontiguousarray(
        x.reshape(B, CIN, WIN, WIN, H, H).transpose(0, 1, 4, 2, 3, 5)[
            :, :, :, :RR, :RR, :
        ]
    ).reshape(B, CIN * H, RCW).astype(np.float16)
    # W: (1, 288, 32, 1, 1, 4, 4); p = cin*9 + kx*3 + ky
    # wt_sb[cin*4+v, kk*128 + o*4+u] = Wm[cin*9+kk, o, u, v]
    Wm = np.asarray(W, dtype=np.float32).reshape(CIN, KK * KK, COUT, H, H)
    wt_sb = np.ascontiguousarray(
        Wm.transpose(0, 4, 1, 2, 3)  # cin, v, kk, o, u
    ).reshape(128, 9 * 128).astype(np.float16)
    bias_v = np.repeat(np.asarray(bias, dtype=np.float32).reshape(COUT), H)
    return xp, wt_sb, bias_v


def _shard_x(xp, core):
    # per-core input: [BPC, 128, RCW] fp16
    return np.ascontiguousarray(xp[core * BPC : (core + 1) * BPC])


def _unprep_output(full, bias_v):
    # full: (B, 128, NOUT) with partition o*4+u, free (X, Y, w)
    full = full.astype(np.float32) + bias_v[None, :, None]
    out = (
        full.reshape(B, COUT, H, WOUT, WOUT, H)
        .transpose(0, 1, 3, 4, 2, 5)
        .reshape(B, COUT, WOUT, WOUT, HH)
    )
    return np.ascontiguousarray(out)


def run_device(in_maps, trace=False, tmpdir=None):
    from concourse.bass_utils import run_bass_kernel_spmd

    if "nc" not in _cache:
        _cache["nc"] = _build_bass()
    return run_bass_kernel_spmd(
        _cache["nc"], in_maps, list(range(NCORES)), trace=trace, tmpdir=tmpdir
    )


def kernel(x, W, bias):
    x = np.asarray(x, dtype=np.float32)
    xp, wt_sb, bias_v = _prep_inputs(x, W, bias)
    in_maps = [{"x": _shard_x(xp, i), "w": wt_sb} for i in range(NCORES)]
    res = run_device(in_maps, trace=False)
    full = np.concatenate(
        [res.results[i]["out"] for i in range(NCORES)], axis=0
    )
    return _unprep_output(full, bias_v)


# revision 23
# speedup vs baseline: 1.0216x; 1.0216x over previous
"""Trainium2 Bass kernel for nn_Conv2dGeneral (capsule-style 4x4-pose conv).

Math (from the reference):
  out[b,o,X,Y,u,w] = sum_{cin,kx,ky,v} Wm[(cin,kx,ky),o,u,v] * x[b,cin,2X+kx,2Y+ky,4v+w] + bias[o]

Mapped to the PE array as a single 1152-deep contraction:
  K = (cin, v)  x  9 accumulation chunks over (kx, ky)   [9 x 128]
  M = (o, u)                                              [128 PSUM partitions]
  N = (X, Y, w)                                           [<=676 per batch]

Data-parallel across 8 NeuronCores on the batch dim (8 images per core).

Host-side prep: x is re-laid-out to [(b), (cin,v), (r,c,w)] with the unused
row/col 27 trimmed (stride-2 K=3 windows over 28 only touch 0..26), so each
core's shard DMAs as contiguous 5.8KB partition lines; the im2col window
gather happens for free inside the matmul moving-operand access pattern.

Scheduling model (measured): the core clock runs at HALF speed except for a
single ~23.9us full-speed budget granted ~7us after the DMA stream starts
(pausing if the PE idles), and the early DMA stream only manages
~150-230GB/s (per-packet overhead ~90ns).  So the critical path is "when
can the PE start" plus "does the post-kernel semaphore sweep still fit in
the full-speed window".  The kernel therefore splits the load across BOTH
HWDGE queues: the SP queue carries weight taps 0-2 then pure x in PE
consumption order (batch 0 in three row-chunks so the first small PSUM
group starts ~2us earlier), while the Scalar queue's ~2us cold-start
swallows weight taps 3-8, which are only needed mid-way into the first
accumulation group.  ACT pre-warms its lazy activation-table load with a
dummy 1-element activation, evicts PSUM->SBUF in fp16, and ships each
X-half right after its eviction so the final drain is one 80KB chunk.

The bias add lives on the host (a 128-descriptor 4-byte-line DMA clogs the
queue for ~1.2us).

Each DMA gets its OWN semaphore and consumers wait for the full +16: the 16
increments of one DMA are sem-update packets distributed round-robin over
the 16 DMA engines in engine-local order, so two DMAs sharing a semaphore
can satisfy a >=16 wait with a mix of packets from both while a slow engine
still has data of the first in flight (cold-first-run corruption).
"""

import numpy as np

B, CIN, COUT = 64, 32, 32
KK, STRIDE = 3, 2
WIN, HH = 28, 16
H = 4
WOUT = (WIN - KK) // STRIDE + 1  # 13
NCORES = 8
BPC = B // NCORES                # batches per core
RR = 2 * WOUT + 1                # 27 rows/cols actually read
RCW = RR * RR * H                # 2916 free elements per (cin,v) partition
NOUT = WOUT * WOUT * H           # 676 outputs per (o,u) partition per image

_cache = {}


def _build_bass():
    """Raw-bass build (no Tile): this toolchain's walrus codegen allows only
    ONE sync-wait per instruction, so all cross-engine sync is explicit
    single-sem waits; ordering beyond that rides on hardware transitivity."""
    import concourse.bass as bass
    import concourse.mybir as mybir
    from contextlib import ExitStack

    f32 = mybir.dt.float32
    f16 = mybir.dt.float16
    WARMUP = 8                # PE warm-up matmuls while x[0] streams in
    HC = WOUT * H             # 52 output columns per X row
    W0E = 3 * 128             # weight taps 0..2 (warmup + first matmuls)
    N0 = 7 * HC               # X-half output widths: 364 / 312
    # batch-0 row chunks: rows [0,5) [5,11) [11,19) [19,27) feeding X groups
    # [0,2) [2,5) [5,9) [9,13)  (chunk boundaries reuse earlier rows in SBUF)
    ROWB = (0, 5, 11, 19, 27)
    X4 = ((0, 2), (2, 3), (5, 4), (9, 4))

    nc = bass.Bass()
    x_d = nc.declare_dram_parameter("x", [BPC, 128, RCW], f16, isOutput=False)
    w_d = nc.declare_dram_parameter("w", [128, 9 * 128], f16, isOutput=False)
    o_d = nc.declare_dram_parameter("out", [BPC, 128, NOUT], f16, isOutput=True)

    with ExitStack() as stack:
        ec = stack.enter_context
        wt = ec(nc.sbuf_tensor([128, 9 * 128], f16))
        gt = ec(nc.sbuf_tensor([128, BPC, RCW], f16))
        ot = ec(nc.sbuf_tensor([128, BPC, NOUT], f16))
        ps = ec(nc.psum_tensor([128, 8, 512], f32))
        wt0_sem = ec(nc.semaphore("wt0_sem"))
        wtr_sem = ec(nc.semaphore("wtr_sem"))
        c_sems = [ec(nc.semaphore(f"c_sem{i}")) for i in range(4)]
        g_sems = {b: ec(nc.semaphore(f"g_sem{b}")) for b in range(1, BPC)}
        pe_sem = ec(nc.semaphore("pe_sem"))
        act_sem = ec(nc.semaphore("act_sem"))
        out_sem = ec(nc.semaphore("out_sem"))
        block = ec(nc.Block())
        wtr = wt[:, :].rearrange("p (k m) -> p k m", k=9)

        # (batch, X0, nX, input-gate sem, ship): batch 0 in four X chunks
        # shipped whole at its end, the rest in two X halves shipped each
        groups = [
            (0, X0, nX, c_sems[c], (0, NOUT) if c == 3 else None)
            for c, (X0, nX) in enumerate(X4)
        ]
        for b in range(1, BPC):
            groups.append((b, 0, 7, g_sems[b], (0, N0)))
            groups.append((b, 7, 6, None, (N0, NOUT - N0)))
        NG = len(groups)  # 18
        NSHIP = 2 * BPC - 1

        @block.sync
        def _(sync):
            # SP queue: taps 0-2 then pure x in PE consumption order
            sync.dma_start(wt[:, :W0E], w_d[:, :W0E]).then_inc(wt0_sem, 16)
            for c in range(4):
                r0, r1 = ROWB[c] * RR * H, ROWB[c + 1] * RR * H
                sync.dma_start(gt[:, 0, r0:r1], x_d[0, :, r0:r1]).then_inc(
                    c_sems[c], 16
                )
            for b in range(1, BPC):
                sync.dma_start(gt[:, b, :], x_d[b]).then_inc(g_sems[b], 16)
            sync.wait_ge(out_sem, 16 * NSHIP)

        @block.tensor
        def _(tensor):
            tensor.wait_ge(wt0_sem, 16)
            # keep PE visibly busy from the start (pulls the clock grant in)
            for i in range(WARMUP):
                tensor.matmul(
                    ps[:, 7, :128], wt[:, :128], wt[:, :128], start=True, stop=True
                )
            for j, (b, X0, nX, gate, ship) in enumerate(groups):
                if gate is not None:
                    tensor.wait_ge(gate, 16)
                if j >= 8:
                    # PSUM bank j%8 is free once group j-8 was evicted
                    tensor.wait_ge(act_sem, j - 7)
                gr = gt[:, b, :].rearrange("p (r c w) -> p r c w", r=RR, c=RR)
                for kk in range(9):
                    if j == 0 and kk == 3:
                        tensor.wait_ge(wtr_sem, 16)  # taps 3-8 land mid-group
                    kx, ky = divmod(kk, 3)
                    rhs = gr[
                        :,
                        2 * X0 + kx : 2 * X0 + kx + 2 * nX - 1 : 2,
                        ky : ky + 2 * WOUT - 1 : 2,
                        :,
                    ]
                    mm = tensor.matmul(
                        ps[:, j % 8, : nX * HC],
                        wtr[:, kk, :],
                        rhs,
                        start=(kk == 0),
                        stop=(kk == 8),
                    )
                mm.then_inc(pe_sem, 1)

        @block.scalar
        def _(scalar):
            # Scalar HWDGE queue: its cold-start swallows taps 3-8, which
            # are only needed mid-way into the first accumulation group
            scalar.dma_start(wt[:, W0E:], w_d[:, W0E:]).then_inc(wtr_sem, 16)
            # pre-warm the lazy activation-table load off the critical path
            scalar.wait_ge(wt0_sem, 16)
            scalar.activation(
                ot[:, 0, :1], wt[:, :1], mybir.ActivationFunctionType.Identity
            )
            for j, (b, X0, nX, gate, ship) in enumerate(groups):
                off = X0 * HC
                scalar.wait_ge(pe_sem, j + 1)
                scalar.activation(
                    ot[:, b, off : off + nX * HC],
                    ps[:, j % 8, : nX * HC],
                    mybir.ActivationFunctionType.Identity,
                ).then_inc(act_sem, 1)
                # ship each completed output range right after its eviction
                if ship is not None:
                    s0, slen = ship
                    scalar.dma_start(
                        o_d[b, :, s0 : s0 + slen], ot[:, b, s0 : s0 + slen]
                    ).then_inc(out_sem, 16)

    return nc


def _prep_inputs(x, W, bias):
    # x: (B, CIN, 28, 28, 16) -> xp[b, cin*4+v, (r*27+c)*4+w] = x[b,cin,r,c,4v+w]
    # (row/col 27 trimmed: stride-2 3-wide windows only read 0..26)
    # fp16 halves the dominant HBM traffic; fp32 PSUM accumulation keeps the
    # 1152-deep contraction at ~5e-4 max rel err.
    xp = np.ascontiguousarray(
        x.reshape(B, CIN, WIN, WIN, H, H).transpose(0, 1, 4, 2, 3, 5)[
            :, :, :, :RR, :RR, :
        ]
    ).reshape(B, CIN * H, RCW).astype(np.float16)
    # W: (1, 288, 32, 1, 1, 4, 4); p = cin*9 + kx*3 + ky
    # wt_sb[cin*4+v, kk*128 + o*4+u] = Wm[cin*9+kk, o, u, v]
    Wm = np.asarray(W, dtype=np.float32).reshape(CIN, KK * KK, COUT, H, H)
    wt_sb = np.ascontiguousarray(
        Wm.transpose(0, 4, 1, 2, 3)  # cin, v, kk, o, u
    ).reshape(128, 9 * 128).astype(np.float16)
    bias_v = np.repeat(np.asarray(bias, dtype=np.float32).reshape(COUT), H)
    return xp, wt_sb, bias_v


def _shard_x(xp, core):
    # per-core input: [BPC, 128, RCW] fp16
    return np.ascontiguousarray(xp[core * BPC : (core + 1) * BPC])


def _unprep_output(full, bias_v):
    # full: (B, 128, NOUT) with partition o*4+u, free (X, Y, w)
    full = full.astype(np.float32) + bias_v[None, :, None]
    out = (
        full.reshape(B, COUT, H, WOUT, WOUT, H)
        .transpose(0, 1, 3, 4, 2, 5)
        .reshape(B, COUT, WOUT, WOUT, HH)
    )
    return np.ascontiguousarray(out)


def run_device(in_maps, trace=False, tmpdir=None):
    from concourse.bass_utils import run_bass_kernel_spmd

    if "nc" not in _cache:
        _cache["nc"] = _build_bass()
    return run_bass_kernel_spmd(
        _cache["nc"], in_maps, list(range(NCORES)), trace=trace, tmpdir=tmpdir
    )


def kernel(x, W, bias):
    x = np.asarray(x, dtype=np.float32)
    xp, wt_sb, bias_v = _prep_inputs(x, W, bias)
    in_maps = [{"x": _shard_x(xp, i), "w": wt_sb} for i in range(NCORES)]
    res = run_device(in_maps, trace=False)
    full = np.concatenate(
        [res.results[i]["out"] for i in range(NCORES)], axis=0
    )
    return _unprep_output(full, bias_v)


# revision 25
# speedup vs baseline: 1.0354x; 1.0136x over previous
"""Trainium2 Bass kernel for nn_Conv2dGeneral (capsule-style 4x4-pose conv).

Math (from the reference):
  out[b,o,X,Y,u,w] = sum_{cin,kx,ky,v} Wm[(cin,kx,ky),o,u,v] * x[b,cin,2X+kx,2Y+ky,4v+w] + bias[o]

Mapped to the PE array as a single 1152-deep contraction:
  K = (cin, v)  x  9 accumulation chunks over (kx, ky)   [9 x 128]
  M = (o, u)                                              [128 PSUM partitions]
  N = (X, Y, w)                                           [<=676 per batch]

Data-parallel across 8 NeuronCores on the batch dim (8 images per core).

Host-side prep: x is re-laid-out to [(b), (cin,v), (r,c,w)] with the unused
row/col 27 trimmed (stride-2 K=3 windows over 28 only touch 0..26), so each
core's shard DMAs as contiguous 5.8KB partition lines; the im2col window
gather happens for free inside the matmul moving-operand access pattern.

Scheduling model (measured): the core clock runs at HALF speed except for a
single ~23.9us full-speed budget granted ~7us after the DMA stream starts
(pausing if the PE idles), and the early DMA stream only manages
~150-230GB/s (per-packet overhead ~90ns).  So the critical path is "when
can the PE start" plus "does the post-kernel semaphore sweep still fit in
the full-speed window".  The kernel therefore splits the load across BOTH
HWDGE queues: the SP queue carries weight taps 0-2 then pure x in PE
consumption order (batch 0 in three row-chunks so the first small PSUM
group starts ~2us earlier), while the Scalar queue's ~2us cold-start
swallows weight taps 3-8, which are only needed mid-way into the first
accumulation group.  ACT pre-warms its lazy activation-table load with a
dummy 1-element activation, evicts PSUM->SBUF in fp16, and ships each
X-half right after its eviction so the final drain is one 80KB chunk.

The bias add lives on the host (a 128-descriptor 4-byte-line DMA clogs the
queue for ~1.2us).

Each DMA gets its OWN semaphore and consumers wait for the full +16: the 16
increments of one DMA are sem-update packets distributed round-robin over
the 16 DMA engines in engine-local order, so two DMAs sharing a semaphore
can satisfy a >=16 wait with a mix of packets from both while a slow engine
still has data of the first in flight (cold-first-run corruption).
"""

import numpy as np

B, CIN, COUT = 64, 32, 32
KK, STRIDE = 3, 2
WIN, HH = 28, 16
H = 4
WOUT = (WIN - KK) // STRIDE + 1  # 13
NCORES = 8
BPC = B // NCORES                # batches per core
RR = 2 * WOUT + 1                # 27 rows/cols actually read
RCW = RR * RR * H                # 2916 free elements per (cin,v) partition
NOUT = WOUT * WOUT * H           # 676 outputs per (o,u) partition per image

_cache = {}


def _build_bass():
    """Raw-bass build (no Tile): this toolchain's walrus codegen allows only
    ONE sync-wait per instruction, so all cross-engine sync is explicit
    single-sem waits; ordering beyond that rides on hardware transitivity."""
    import concourse.bass as bass
    import concourse.mybir as mybir
    from contextlib import ExitStack

    f32 = mybir.dt.float32
    f16 = mybir.dt.float16
    WARMUP = 5                # PE warm-up matmuls while x[0] streams in
    HC = WOUT * H             # 52 output columns per X row
    W0E = 3 * 128             # weight taps 0..2 (warmup + first matmuls)
    N0 = 7 * HC               # X-half output widths: 364 / 312
    # batch-0 row chunks: rows [0,9) [9,15) [15,27) feeding X groups
    # [0,4) [4,7) [7,13)  (chunk boundaries reuse earlier rows in SBUF;
    # finer chunks lose more to ~100ns/packet DMA-engine overhead than the
    # earlier PE start gains)
    ROWB = (0, 9, 15, 27)
    X4 = ((0, 4), (4, 3), (7, 6))

    nc = bass.Bass()
    x_d = nc.declare_dram_parameter("x", [BPC, 128, RCW], f16, isOutput=False)
    w_d = nc.declare_dram_parameter("w", [128, 9 * 128], f16, isOutput=False)
    o_d = nc.declare_dram_parameter("out", [BPC, 128, NOUT], f16, isOutput=True)

    with ExitStack() as stack:
        ec = stack.enter_context
        wt = ec(nc.sbuf_tensor([128, 9 * 128], f16))
        gt = ec(nc.sbuf_tensor([128, BPC, RCW], f16))
        ot = ec(nc.sbuf_tensor([128, BPC, NOUT], f16))
        ps = ec(nc.psum_tensor([128, 8, 512], f32))
        wt0_sem = ec(nc.semaphore("wt0_sem"))
        wtr_sem = ec(nc.semaphore("wtr_sem"))
        c_sems = [ec(nc.semaphore(f"c_sem{i}")) for i in range(4)]
        g_sems = {b: ec(nc.semaphore(f"g_sem{b}")) for b in range(1, BPC)}
        pe_sem = ec(nc.semaphore("pe_sem"))
        act_sem = ec(nc.semaphore("act_sem"))
        out_sem = ec(nc.semaphore("out_sem"))
        block = ec(nc.Block())
        wtr = wt[:, :].rearrange("p (k m) -> p k m", k=9)

        # (batch, X0, nX, input-gate sem, ship): batch 0 in three X chunks,
        # the rest in two X halves; ship each completed X-half
        groups = [
            (0, 0, 4, c_sems[0], None),
            (0, 4, 3, c_sems[1], (0, N0)),
            (0, 7, 6, c_sems[2], (N0, NOUT - N0)),
        ]
        for b in range(1, BPC):
            groups.append((b, 0, 7, g_sems[b], (0, N0)))
            groups.append((b, 7, 6, None, (N0, NOUT - N0)))
        NG = len(groups)  # 17
        NSHIP = 2 * BPC

        @block.sync
        def _(sync):
            # SP queue: taps 0-2 then pure x in PE consumption order
            sync.dma_start(wt[:, :W0E], w_d[:, :W0E]).then_inc(wt0_sem, 16)
            for c in range(3):
                r0, r1 = ROWB[c] * RR * H, ROWB[c + 1] * RR * H
                sync.dma_start(gt[:, 0, r0:r1], x_d[0, :, r0:r1]).then_inc(
                    c_sems[c], 16
                )
            for b in range(1, BPC):
                sync.dma_start(gt[:, b, :], x_d[b]).then_inc(g_sems[b], 16)
            sync.wait_ge(out_sem, 16 * NSHIP)

        @block.tensor
        def _(tensor):
            tensor.wait_ge(wt0_sem, 16)
            # keep PE visibly busy from the start (pulls the clock grant in)
            for i in range(WARMUP):
                tensor.matmul(
                    ps[:, 7, :128], wt[:, :128], wt[:, :128], start=True, stop=True
                )
            for j, (b, X0, nX, gate, ship) in enumerate(groups):
                if gate is not None:
                    tensor.wait_ge(gate, 16)
                if j >= 8:
                    # PSUM bank j%8 is free once group j-8 was evicted
                    tensor.wait_ge(act_sem, j - 7)
                gr = gt[:, b, :].rearrange("p (r c w) -> p r c w", r=RR, c=RR)
                for kk in range(9):
                    if j == 0 and kk == 3:
                        tensor.wait_ge(wtr_sem, 16)  # taps 3-8 land mid-group
                    kx, ky = divmod(kk, 3)
                    rhs = gr[
                        :,
                        2 * X0 + kx : 2 * X0 + kx + 2 * nX - 1 : 2,
                        ky : ky + 2 * WOUT - 1 : 2,
                        :,
                    ]
                    mm = tensor.matmul(
                        ps[:, j % 8, : nX * HC],
                        wtr[:, kk, :],
                        rhs,
                        start=(kk == 0),
                        stop=(kk == 8),
                    )
                mm.then_inc(pe_sem, 1)

        @block.scalar
        def _(scalar):
            # Scalar HWDGE queue: its cold-start swallows taps 3-8, which
            # are only needed mid-way into the first accumulation group
            scalar.dma_start(wt[:, W0E:], w_d[:, W0E:]).then_inc(wtr_sem, 16)
            # pre-warm the lazy activation-table load off the critical path
            scalar.wait_ge(wt0_sem, 16)
            scalar.activation(
                ot[:, 0, :1], wt[:, :1], mybir.ActivationFunctionType.Identity
            )
            for j, (b, X0, nX, gate, ship) in enumerate(groups):
                off = X0 * HC
                scalar.wait_ge(pe_sem, j + 1)
                scalar.activation(
                    ot[:, b, off : off + nX * HC],
                    ps[:, j % 8, : nX * HC],
                    mybir.ActivationFunctionType.Identity,
                ).then_inc(act_sem, 1)
                # ship each completed output range right after its eviction
                if ship is not None:
                    s0, slen = ship
                    scalar.dma_start(
                        o_d[b, :, s0 : s0 + slen], ot[:, b, s0 : s0 + slen]
                    ).then_inc(out_sem, 16)

    return nc


def _prep_inputs(x, W, bias):
    # x: (B, CIN, 28, 28, 16) -> xp[b, cin*4+v, (r*27+c)*4+w] = x[b,cin,r,c,4v+w]
    # (row/col 27 trimmed: stride-2 3-wide windows only read 0..26)
    # fp16 halves the dominant HBM traffic; fp32 PSUM accumulation keeps the
    # 1152-deep contraction at ~5e-4 max rel err.
    xp = np.ascontiguousarray(
        x.reshape(B, CIN, WIN, WIN, H, H).transpose(0, 1, 4, 2, 3, 5)[
            :, :, :, :RR, :RR, :
        ]
    ).reshape(B, CIN * H, RCW).astype(np.float16)
    # W: (1, 288, 32, 1, 1, 4, 4); p = cin*9 + kx*3 + ky
    # wt_sb[cin*4+v, kk*128 + o*4+u] = Wm[cin*9+kk, o, u, v]
    Wm = np.asarray(W, dtype=np.float32).reshape(CIN, KK * KK, COUT, H, H)
    wt_sb = np.ascontiguousarray(
        Wm.transpose(0, 4, 1, 2, 3)  # cin, v, kk, o, u
    ).reshape(128, 9 * 128).astype(np.float16)
    bias_v = np.repeat(np.asarray(bias, dtype=np.float32).reshape(COUT), H)
    return xp, wt_sb, bias_v


def _shard_x(xp, core):
    # per-core input: [BPC, 128, RCW] fp16
    return np.ascontiguousarray(xp[core * BPC : (core + 1) * BPC])


def _unprep_output(full, bias_v):
    # full: (B, 128, NOUT) with partition o*4+u, free (X, Y, w)
    full = full.astype(np.float32) + bias_v[None, :, None]
    out = (
        full.reshape(B, COUT, H, WOUT, WOUT, H)
        .transpose(0, 1, 3, 4, 2, 5)
        .reshape(B, COUT, WOUT, WOUT, HH)
    )
    return np.ascontiguousarray(out)


def run_device(in_maps, trace=False, tmpdir=None):
    from concourse.bass_utils import run_bass_kernel_spmd

    if "nc" not in _cache:
        _cache["nc"] = _build_bass()
    return run_bass_kernel_spmd(
        _cache["nc"], in_maps, list(range(NCORES)), trace=trace, tmpdir=tmpdir
    )


def kernel(x, W, bias):
    x = np.asarray(x, dtype=np.float32)
    xp, wt_sb, bias_v = _prep_inputs(x, W, bias)
    in_maps = [{"x": _shard_x(xp, i), "w": wt_sb} for i in range(NCORES)]
    res = run_device(in_maps, trace=False)
    full = np.concatenate(
        [res.results[i]["out"] for i in range(NCORES)], axis=0
    )
    return _unprep_output(full, bias_v)


# revision 27
# speedup vs baseline: 1.0441x; 1.0083x over previous
"""Trainium2 Bass kernel for nn_Conv2dGeneral (capsule-style 4x4-pose conv).

Math (from the reference):
  out[b,o,X,Y,u,w] = sum_{cin,kx,ky,v} Wm[(cin,kx,ky),o,u,v] * x[b,cin,2X+kx,2Y+ky,4v+w] + bias[o]

Mapped to the PE array as a single 1152-deep contraction:
  K = (cin, v)  x  9 accumulation chunks over (kx, ky)   [9 x 128]
  M = (o, u)                                              [128 PSUM partitions]
  N = (X, Y, w)                                           [<=676 per batch]

Data-parallel across 8 NeuronCores on the batch dim (8 images per core).

Host-side prep: x is re-laid-out to [(b), (cin,v), (r,c,w)] with the unused
row/col 27 trimmed (stride-2 K=3 windows over 28 only touch 0..26), so each
core's shard DMAs as contiguous 5.8KB partition lines; the im2col window
gather happens for free inside the matmul moving-operand access pattern.

Scheduling model (measured): the core clock runs at HALF speed except for a
single ~23.9us full-speed budget granted ~7us after the DMA stream starts
(pausing if the PE idles), and the early DMA stream only manages
~150-230GB/s (per-packet overhead ~90ns).  So the critical path is "when
can the PE start" plus "does the post-kernel semaphore sweep still fit in
the full-speed window".  The kernel therefore splits the load across BOTH
HWDGE queues: the SP queue carries weight taps 0-2 then pure x in PE
consumption order (batch 0 in three row-chunks so the first small PSUM
group starts ~2us earlier), while the Scalar queue's ~2us cold-start
swallows weight taps 3-8, which are only needed mid-way into the first
accumulation group.  ACT pre-warms its lazy activation-table load with a
dummy 1-element activation, evicts PSUM->SBUF in fp16, and ships each
X-half right after its eviction so the final drain is one 80KB chunk.

The bias add lives on the host (a 128-descriptor 4-byte-line DMA clogs the
queue for ~1.2us).

Each DMA gets its OWN semaphore and consumers wait for the full +16: the 16
increments of one DMA are sem-update packets distributed round-robin over
the 16 DMA engines in engine-local order, so two DMAs sharing a semaphore
can satisfy a >=16 wait with a mix of packets from both while a slow engine
still has data of the first in flight (cold-first-run corruption).
"""

import numpy as np

B, CIN, COUT = 64, 32, 32
KK, STRIDE = 3, 2
WIN, HH = 28, 16
H = 4
WOUT = (WIN - KK) // STRIDE + 1  # 13
NCORES = 8
BPC = B // NCORES                # batches per core
RR = 2 * WOUT + 1                # 27 rows/cols actually read
RCW = RR * RR * H                # 2916 free elements per (cin,v) partition
NOUT = WOUT * WOUT * H           # 676 outputs per (o,u) partition per image

_cache = {}


def _build_bass():
    """Raw-bass build (no Tile): this toolchain's walrus codegen allows only
    ONE sync-wait per instruction, so all cross-engine sync is explicit
    single-sem waits; ordering beyond that rides on hardware transitivity."""
    import concourse.bass as bass
    import concourse.mybir as mybir
    from contextlib import ExitStack

    f32 = mybir.dt.float32
    f16 = mybir.dt.float16
    WARMUP = 5                # PE warm-up matmuls while x[0] streams in
    HC = WOUT * H             # 52 output columns per X row
    W0E = 3 * 128             # weight taps 0..2 (warmup + first matmuls)
    N0 = 7 * HC               # X-half output widths: 364 / 312
    # batch-0 row chunks: rows [0,7) [7,15) [15,27) feeding X groups
    # [0,3) [3,7) [7,13)  (chunk boundaries reuse earlier rows in SBUF;
    # much finer chunks lose more to ~100ns/packet DMA-engine overhead than
    # the earlier PE start gains)
    ROWB = (0, 7, 15, 27)
    X4 = ((0, 3), (3, 4), (7, 6))

    nc = bass.Bass()
    x_d = nc.declare_dram_parameter("x", [BPC, 128, RCW], f16, isOutput=False)
    w_d = nc.declare_dram_parameter("w", [128, 9 * 128], f16, isOutput=False)
    o_d = nc.declare_dram_parameter("out", [BPC, 128, NOUT], f16, isOutput=True)

    with ExitStack() as stack:
        ec = stack.enter_context
        wt = ec(nc.sbuf_tensor([128, 9 * 128], f16))
        gt = ec(nc.sbuf_tensor([128, BPC, RCW], f16))
        ot = ec(nc.sbuf_tensor([128, BPC, NOUT], f16))
        ps = ec(nc.psum_tensor([128, 8, 512], f32))
        wt0_sem = ec(nc.semaphore("wt0_sem"))
        wtr_sem = ec(nc.semaphore("wtr_sem"))
        c_sems = [ec(nc.semaphore(f"c_sem{i}")) for i in range(4)]
        g_sems = {b: ec(nc.semaphore(f"g_sem{b}")) for b in range(1, BPC)}
        pe_sem = ec(nc.semaphore("pe_sem"))
        act_sem = ec(nc.semaphore("act_sem"))
        out_sem = ec(nc.semaphore("out_sem"))
        block = ec(nc.Block())
        wtr = wt[:, :].rearrange("p (k m) -> p k m", k=9)

        # (batch, X0, nX, input-gate sem, ship): batch 0 in three X chunks,
        # the rest in two X halves; ship each completed X-half
        groups = [
            (0, 0, 3, c_sems[0], None),
            (0, 3, 4, c_sems[1], (0, N0)),
            (0, 7, 6, c_sems[2], (N0, NOUT - N0)),
        ]
        for b in range(1, BPC):
            groups.append((b, 0, 7, g_sems[b], (0, N0)))
            groups.append((b, 7, 6, None, (N0, NOUT - N0)))
        NG = len(groups)  # 17
        NSHIP = 2 * BPC

        @block.sync
        def _(sync):
            # SP queue: taps 0-2 then pure x in PE consumption order
            sync.dma_start(wt[:, :W0E], w_d[:, :W0E]).then_inc(wt0_sem, 16)
            for c in range(3):
                r0, r1 = ROWB[c] * RR * H, ROWB[c + 1] * RR * H
                sync.dma_start(gt[:, 0, r0:r1], x_d[0, :, r0:r1]).then_inc(
                    c_sems[c], 16
                )
            for b in range(1, BPC):
                sync.dma_start(gt[:, b, :], x_d[b]).then_inc(g_sems[b], 16)
            sync.wait_ge(out_sem, 16 * NSHIP)

        @block.tensor
        def _(tensor):
            tensor.wait_ge(wt0_sem, 16)
            # keep PE visibly busy from the start (pulls the clock grant in)
            for i in range(WARMUP):
                tensor.matmul(
                    ps[:, 7, :128], wt[:, :128], wt[:, :128], start=True, stop=True
                )
            for j, (b, X0, nX, gate, ship) in enumerate(groups):
                if gate is not None:
                    tensor.wait_ge(gate, 16)
                if j >= 8:
                    # PSUM bank j%8 is free once group j-8 was evicted
                    tensor.wait_ge(act_sem, j - 7)
                gr = gt[:, b, :].rearrange("p (r c w) -> p r c w", r=RR, c=RR)
                for kk in range(9):
                    if j == 0 and kk == 3:
                        tensor.wait_ge(wtr_sem, 16)  # taps 3-8 land mid-group
                    kx, ky = divmod(kk, 3)
                    rhs = gr[
                        :,
                        2 * X0 + kx : 2 * X0 + kx + 2 * nX - 1 : 2,
                        ky : ky + 2 * WOUT - 1 : 2,
                        :,
                    ]
                    mm = tensor.matmul(
                        ps[:, j % 8, : nX * HC],
                        wtr[:, kk, :],
                        rhs,
                        start=(kk == 0),
                        stop=(kk == 8),
                    )
                mm.then_inc(pe_sem, 1)

        @block.scalar
        def _(scalar):
            # Scalar HWDGE queue: its cold-start swallows taps 3-8, which
            # are only needed mid-way into the first accumulation group
            scalar.dma_start(wt[:, W0E:], w_d[:, W0E:]).then_inc(wtr_sem, 16)
            # pre-warm the lazy activation-table load off the critical path
            scalar.wait_ge(wt0_sem, 16)
            scalar.activation(
                ot[:, 0, :1], wt[:, :1], mybir.ActivationFunctionType.Identity
            )
            for j, (b, X0, nX, gate, ship) in enumerate(groups):
                off = X0 * HC
                scalar.wait_ge(pe_sem, j + 1)
                scalar.activation(
                    ot[:, b, off : off + nX * HC],
                    ps[:, j % 8, : nX * HC],
                    mybir.ActivationFunctionType.Identity,
                ).then_inc(act_sem, 1)
                # ship each completed output range right after its eviction
                if ship is not None:
                    s0, slen = ship
                    scalar.dma_start(
                        o_d[b, :, s0 : s0 + slen], ot[:, b, s0 : s0 + slen]
                    ).then_inc(out_sem, 16)

    return nc


def _prep_inputs(x, W, bias):
    # x: (B, CIN, 28, 28, 16) -> xp[b, cin*4+v, (r*27+c)*4+w] = x[b,cin,r,c,4v+w]
    # (row/col 27 trimmed: stride-2 3-wide windows only read 0..26)
    # fp16 halves the dominant HBM traffic; fp32 PSUM accumulation keeps the
    # 1152-deep contraction at ~5e-4 max rel err.
    xp = np.ascontiguousarray(
        x.reshape(B, CIN, WIN, WIN, H, H).transpose(0, 1, 4, 2, 3, 5)[
            :, :, :, :RR, :RR, :
        ]
    ).reshape(B, CIN * H, RCW).astype(np.float16)
    # W: (1, 288, 32, 1, 1, 4, 4); p = cin*9 + kx*3 + ky
    # wt_sb[cin*4+v, kk*128 + o*4+u] = Wm[cin*9+kk, o, u, v]
    Wm = np.asarray(W, dtype=np.float32).reshape(CIN, KK * KK, COUT, H, H)
    wt_sb = np.ascontiguousarray(
        Wm.transpose(0, 4, 1, 2, 3)  # cin, v, kk, o, u
    ).reshape(128, 9 * 128).astype(np.float16)
    bias_v = np.repeat(np.asarray(bias, dtype=np.float32).reshape(COUT), H)
    return xp, wt_sb, bias_v


def _shard_x(xp, core):
    # per-core input: [BPC, 128, RCW] fp16
    return np.ascontiguousarray(xp[core * BPC : (core + 1) * BPC])


def _unprep_output(full, bias_v):
    # full: (B, 128, NOUT) with partition o*4+u, free (X, Y, w)
    full = full.astype(np.float32) + bias_v[None, :, None]
    out = (
        full.reshape(B, COUT, H, WOUT, WOUT, H)
        .transpose(0, 1, 3, 4, 2, 5)
        .reshape(B, COUT, WOUT, WOUT, HH)
    )
    return np.ascontiguousarray(out)


def run_device(in_maps, trace=False, tmpdir=None):
    from concourse.bass_utils import run_bass_kernel_spmd

    if "nc" not in _cache:
        _cache["nc"] = _build_bass()
    return run_bass_kernel_spmd(
        _cache["nc"], in_maps, list(range(NCORES)), trace=trace, tmpdir=tmpdir
    )


def kernel(x, W, bias):
    x = np.asarray(x, dtype=np.float32)
    xp, wt_sb, bias_v = _prep_inputs(x, W, bias)
    in_maps = [{"x": _shard_x(xp, i), "w": wt_sb} for i in range(NCORES)]
    res = run_device(in_maps, trace=False)
    full = np.concatenate(
        [res.results[i]["out"] for i in range(NCORES)], axis=0
    )
    return _unprep_output(full, bias_v)
